# revision 11
# baseline (speedup 1.0000x reference)
"""GCN layer kernel for Trainium2, 8 NeuronCores (SPMD).

Math (see reference): lap = D^-1/2 A^T D^-1/2 with A = scatter(edges) + I,
ax = lap @ x, out = tanh(ax[:, :, None] * w).

Rewritten as: ax[i, t] = d[i] * sum_j A[j, i] * d[j] * x[j, t], with
d = rowsum(A)^-1/2. Sharded over i (node/dst dim): core c owns rows
[512c, 512c+512).

Device algorithm per core (no dense A materialization):
  - rowsum over the core's 512 src rows via one-hot-mask matmuls over the
    src-binned edge list (self-loops included as w=1 edges), AllGather -> d.
  - y = d * x (rows), write to DRAM, dma_gather y rows per dst-binned edge.
  - Z[i, t] = sum_e w_e * [dst_e == i] * y[src_e, t] via one-hot-mask
    matmuls (mask = (iota == dstslot) * w as lhsT, gathered y rows as rhs).
  - out = tanh((Z * w_k) * d_i), written directly to the output shard.

Host side only dedups/bins/pads the edge list (sharding + DMA marshalling);
all FLOPs and all O(N*T*D) work happen on device.
"""
import sys

sys.path.insert(0, "/opt/trn_rl_repo")

import numpy as np

N = 4096
T = 128
D = 64
NCORES = 8
BLK = N // NCORES          # 512 dst rows per core
NB = 4                     # bins (128-row sub-blocks) per core
NCH = 40                   # chunks of 128 edges per bin
CAP = NCH * 128            # 5120 edge slots per bin
TOTCH = NB * NCH           # 160 chunks per core
TOT = NB * CAP             # 20480 edge slots per core
UNIT = 1024                # idxs per dma_gather (HW ring limit is ~1.5k)
UPB = CAP // UNIT          # 5 gather units per bin
IDXU = UNIT // 16          # 64 idx columns per unit (16-partition wrap)
NUNITS = NB * UPB          # 20 gather units per core

_compiled = None


def _build(use_gather=True, use_cc=True):
    import concourse.bass as bass
    import concourse.mybir as mybir
    import concourse.tile as tile
    from concourse import bacc
    from concourse.bass import ts

    f32 = mybir.dt.float32
    nc = bacc.Bacc(None, target_bir_lowering=False, debug=False)

    x_in = nc.dram_tensor("x", [N, T], f32, kind="ExternalInput")
    w_in = nc.dram_tensor("wvec", [1, D], f32, kind="ExternalInput")
    iota_in = nc.dram_tensor("iota", [1, 128], f32, kind="ExternalInput")
    gidx_in = nc.dram_tensor("gidx", [128, NUNITS * IDXU], mybir.dt.int16,
                             kind="ExternalInput")
    dslot_in = nc.dram_tensor("dslot", [128, TOTCH], f32, kind="ExternalInput")
    dw_in = nc.dram_tensor("dw", [128, TOTCH], f32, kind="ExternalInput")
    sslot_in = nc.dram_tensor("sslot", [128, TOTCH], f32, kind="ExternalInput")
    sw_in = nc.dram_tensor("sw", [128, TOTCH], f32, kind="ExternalInput")
    out_dram = nc.dram_tensor("out", [BLK, T, D], f32, kind="ExternalOutput")

    y_dram = nc.dram_tensor("y_dram", [N, T], f32)
    ag_in = nc.dram_tensor("ag_in", [BLK, 1], f32)
    ag_out = nc.dram_tensor("ag_out", [N, 1], f32)

    with tile.TileContext(nc) as tc:
        with (
            tc.tile_pool(name="const", bufs=1) as constp,
            tc.tile_pool(name="edges", bufs=1) as edgep,
            tc.tile_pool(name="xy", bufs=1) as xyp,
            tc.tile_pool(name="mask", bufs=6) as maskp,
            tc.tile_pool(name="yg", bufs=4) as ygp,
            tc.tile_pool(name="dsm", bufs=1) as dsmp,
            tc.tile_pool(name="ep", bufs=3) as epp,
            tc.tile_pool(name="ep2", bufs=3) as ep2p,
            tc.tile_pool(name="psr", bufs=4, space="PSUM") as psr,
            tc.tile_pool(name="psm", bufs=4, space="PSUM") as psm,
        ):
            # constants
            iota_f = constp.tile([128, 128], f32)
            nc.sync.dma_start(out=iota_f[:], in_=iota_in[0:1, :].to_broadcast([128, 128]))
            wb = constp.tile([128, D], f32)
            nc.sync.dma_start(out=wb[:], in_=w_in[0:1, :].to_broadcast([128, D]))
            ones = constp.tile([128, 1], f32)
            nc.vector.memset(ones[:], 1.0)

            # edge metadata loads
            sslot_sb = edgep.tile([128, TOTCH], f32)
            nc.sync.dma_start(out=sslot_sb[:], in_=sslot_in[:, :])
            sw_sb = edgep.tile([128, TOTCH], f32)
            nc.sync.dma_start(out=sw_sb[:], in_=sw_in[:, :])
            dslot_sb = edgep.tile([128, TOTCH], f32)
            nc.sync.dma_start(out=dslot_sb[:], in_=dslot_in[:, :])
            dw_sb = edgep.tile([128, TOTCH], f32)
            nc.sync.dma_start(out=dw_sb[:], in_=dw_in[:, :])
            gidx_sb = edgep.tile([128, NUNITS * IDXU], mybir.dt.int16)
            nc.sync.dma_start(out=gidx_sb[:], in_=gidx_in[:, :])

            # full x: [128, 32*128], partition p holds rows {128k+p}
            x_sb = xyp.tile([128, N // 128, T], f32)
            nc.sync.dma_start(
                out=x_sb[:],
                in_=x_in[:, :].rearrange("(k p) t -> p k t", p=128),
            )

            # ---- rowsum of A over this core's 512 src rows ----
            rs_sb = dsmp.tile([128, NB], f32)
            for m in range(NB):
                ps = psr.tile([128, 1], f32, tag="rs")
                for ch in range(NCH):
                    col = m * NCH + ch
                    mask = maskp.tile([128, 128], f32, tag="mk")
                    nc.vector.tensor_scalar(
                        out=mask[:], in0=iota_f[:],
                        scalar1=sslot_sb[:, col:col + 1],
                        scalar2=sw_sb[:, col:col + 1],
                        op0=mybir.AluOpType.is_equal,
                        op1=mybir.AluOpType.mult,
                    )
                    nc.tensor.matmul(ps[:], lhsT=mask[:], rhs=ones[:],
                                     start=(ch == 0), stop=(ch == NCH - 1))
                nc.vector.tensor_copy(rs_sb[:, m:m + 1], ps[:])

            # d_local = 1/sqrt(rowsum)
            dl_s = dsmp.tile([128, NB], f32)
            nc.scalar.sqrt(dl_s[:], rs_sb[:])
            d_local = dsmp.tile([128, NB], f32)
            nc.vector.reciprocal(d_local[:], dl_s[:])

            # AllGather d: core c's block -> ag_out[512c : 512c+512]
            nc.sync.dma_start(
                out=ag_in[:, 0].rearrange("(m p) -> p m", p=128),
                in_=d_local[:],
            )
            if use_cc:
                nc.gpsimd.collective_compute(
                    "AllGather", mybir.AluOpType.bypass,
                    replica_groups=[list(range(NCORES))],
                    ins=[ag_in[:, :]], outs=[ag_out[:, :]],
                )
            else:
                nc.sync.dma_start(
                    out=ag_out[0:BLK, :],
                    in_=ag_in[:, :])
            d_sb = dsmp.tile([128, N // 128], f32)
            nc.sync.dma_start(
                out=d_sb[:],
                in_=ag_out[:, 0].rearrange("(k p) -> p k", p=128),
            )

            # y = d * x (row scaling), then to DRAM for the gathers
            y_sb = xyp.tile([128, N // 128, T], f32)
            for k in range(N // 128):
                nc.vector.tensor_scalar_mul(
                    y_sb[:, k, :], x_sb[:, k, :], d_sb[:, k:k + 1])
            nc.sync.dma_start(
                out=y_dram[:, :].rearrange("(k p) t -> p k t", p=128),
                in_=y_sb[:],
            )

            # per-bin gather of y rows + masked matmul + epilogue
            out_r = out_dram[:, :, :].rearrange("(m p) t d -> m p t d", p=128)
            for m in range(NB):
                pm = psm.tile([128, T], f32, tag="mm")
                for v in range(UPB):
                    u = m * UPB + v
                    yg = ygp.tile([128, UNIT // 128, T], f32, tag="yg")
                    if use_gather:
                        nc.gpsimd.dma_gather(
                            out_ap=yg[:],
                            in_ap=y_dram[:, :],
                            idxs_ap=gidx_sb[:, u * IDXU:(u + 1) * IDXU],
                            num_idxs=UNIT,
                            num_idxs_reg=UNIT,
                            elem_size=T,
                        )
                    else:
                        nc.vector.memset(yg[:], 0.0)
                    for i in range(UNIT // 128):
                        ch = v * (UNIT // 128) + i
                        col = m * NCH + ch
                        mask = maskp.tile([128, 128], f32, tag="mk")
                        nc.vector.tensor_scalar(
                            out=mask[:], in0=iota_f[:],
                            scalar1=dslot_sb[:, col:col + 1],
                            scalar2=dw_sb[:, col:col + 1],
                            op0=mybir.AluOpType.is_equal,
                            op1=mybir.AluOpType.mult,
                        )
                        nc.tensor.matmul(pm[:], lhsT=mask[:], rhs=yg[:, i, :],
                                         start=(ch == 0), stop=(ch == NCH - 1))
                # out[i, t, k] = tanh(Z[i, t] * w[k] * d_local[i])
                for tq in range(4):
                    prod = epp.tile([128, T // 4, D], f32, tag="ep")
                    nc.vector.tensor_tensor(
                        out=prod[:],
                        in0=pm[:, ts(tq, T // 4), None].to_broadcast(
                            [128, T // 4, D]),
                        in1=wb[:, None, :].to_broadcast([128, T // 4, D]),
                        op=mybir.AluOpType.mult,
                    )
                    tout = ep2p.tile([128, T // 4, D], f32, tag="ep2")
                    nc.scalar.activation(
                        tout[:], prod[:], mybir.ActivationFunctionType.Tanh,
                        scale=d_local[:, m:m + 1],
                    )
                    nc.scalar.dma_start(
                        out=out_r[m, :, ts(tq, T // 4), :], in_=tout[:])
    nc.compile()
    return nc


def _get_compiled():
    global _compiled
    if _compiled is None:
        _compiled = _build()
    return _compiled


def _marshal(inputs, edge_index, edge_weight, weights):
    x = np.ascontiguousarray(np.asarray(inputs, dtype=np.float32))
    ei = np.asarray(edge_index)
    w = np.asarray(edge_weight, dtype=np.float32)
    wvec = np.ascontiguousarray(np.asarray(weights, dtype=np.float32))
    src = ei[0].astype(np.int64)
    dst = ei[1].astype(np.int64)

    # Dedup matching this backend's .at[src, dst].set(w) semantics: the
    # compiled scatter processes the edge list as 128 parallel chunks of
    # E/128, position-within-chunk ascending, so for duplicate (src, dst)
    # the edge with the largest (e mod E/128) wins (verified exhaustively
    # against the jax scatter for all 553 duplicate cells).
    E = len(w)
    mod = max(1, E // 128)
    e = np.arange(E)
    perm = np.lexsort((e, e % mod))
    key = (src * N + dst)[perm]
    _, first_rev = np.unique(key[::-1], return_index=True)
    keep = perm[E - 1 - first_rev]
    src, dst, w = src[keep], dst[keep], w[keep]

    # self-loops (adj + I) as ordinary edges; a real (i, i) edge coexists
    # with its self-loop and the matmul accumulation adds them: w + 1.
    loop = np.arange(N, dtype=np.int64)
    src = np.concatenate([src, loop])
    dst = np.concatenate([dst, loop])
    w = np.concatenate([w, np.ones(N, np.float32)])

    iota = np.arange(128, dtype=np.float32)[None, :]

    in_maps = []
    for c in range(NCORES):
        gidx = np.zeros((NB, CAP), np.int16)
        dslot = np.zeros((NB, CAP), np.float32)
        dwv = np.zeros((NB, CAP), np.float32)
        sslot = np.zeros((NB, CAP), np.float32)
        swv = np.zeros((NB, CAP), np.float32)
        for m in range(NB):
            lo = c * BLK + m * 128
            e = np.nonzero((dst >= lo) & (dst < lo + 128))[0]
            cnt = len(e)
            assert cnt <= CAP, f"dst bin overflow: {cnt} > {CAP}"
            gidx[m, :cnt] = src[e].astype(np.int16)
            dslot[m, :cnt] = (dst[e] - lo).astype(np.float32)
            dwv[m, :cnt] = w[e]
            es = np.nonzero((src >= lo) & (src < lo + 128))[0]
            cnt = len(es)
            assert cnt <= CAP, f"src bin overflow: {cnt} > {CAP}"
            sslot[m, :cnt] = (src[es] - lo).astype(np.float32)
            swv[m, :cnt] = w[es]

        # gather unit u = m*UPB + v covers entries [v*UNIT, (v+1)*UNIT) of
        # bin m; within a unit, entry e -> gidx_w[e % 16, u*IDXU + e // 16].
        # The 16-row block is replicated across all 8 GpSimd cores' groups.
        gidx_w = np.zeros((128, NUNITS * IDXU), np.int16)
        for m in range(NB):
            for v in range(UPB):
                u = m * UPB + v
                seg = gidx[m, v * UNIT:(v + 1) * UNIT]
                blk16 = seg.reshape(IDXU, 16).T
                gidx_w[:, u * IDXU:(u + 1) * IDXU] = np.tile(blk16, (8, 1))
        # entry e of bin m -> [e % 128, m*NCH + e // 128]
        def chunked(a):
            return np.ascontiguousarray(
                a.reshape(NB * NCH, 128).T.astype(np.float32))

        in_maps.append({
            "x": x,
            "wvec": wvec,
            "iota": iota,
            "gidx": np.ascontiguousarray(gidx_w),
            "dslot": chunked(dslot),
            "dw": chunked(dwv),
            "sslot": chunked(sslot),
            "sw": chunked(swv),
        })
    return in_maps


def kernel(inputs, edge_index, edge_weight, weights):
    from concourse.bass_utils import run_bass_kernel_spmd

    nc = _get_compiled()
    in_maps = _marshal(inputs, edge_index, edge_weight, weights)
    res = run_bass_kernel_spmd(nc, in_maps, core_ids=list(range(NCORES)))
    return np.concatenate([res.results[c]["out"] for c in range(NCORES)], axis=0)


# revision 12
# speedup vs baseline: 1.1846x; 1.1846x over previous
"""GCN layer kernel for Trainium2, 8 NeuronCores (SPMD).

Math (see reference): lap = D^-1/2 A^T D^-1/2 with A = scatter(edges) + I,
ax = lap @ x, out = tanh(ax[:, :, None] * w).

Rewritten as: ax[i, t] = d[i] * sum_j A[j, i] * d[j] * x[j, t], with
d = rowsum(A)^-1/2. Sharded over i (node/dst dim): core c owns rows
[512c, 512c+512).

Device algorithm per core (no dense A materialization):
  - rowsum over the core's 512 src rows via one-hot-mask matmuls over the
    src-binned edge list (self-loops included as w=1 edges), AllGather -> d.
  - y = d * x (rows), write to DRAM, dma_gather y rows per dst-binned edge.
  - Z[i, t] = sum_e w_e * [dst_e == i] * y[src_e, t] via one-hot-mask
    matmuls (mask = (iota == dstslot) * w as lhsT, gathered y rows as rhs).
  - out = tanh((Z * w_k) * d_i), written directly to the output shard.

Host side only dedups/bins/pads the edge list (sharding + DMA marshalling);
all FLOPs and all O(N*T*D) work happen on device.
"""
import sys

sys.path.insert(0, "/opt/trn_rl_repo")

import numpy as np

N = 4096
T = 128
D = 64
NCORES = 8
BLK = N // NCORES          # 512 dst rows per core
NB = 4                     # bins (128-row sub-blocks) per core
NCH = 40                   # chunks of 128 edges per bin
CAP = NCH * 128            # 5120 edge slots per bin
TOTCH = NB * NCH           # 160 chunks per core
TOT = NB * CAP             # 20480 edge slots per core
UNIT = 1024                # idxs per dma_gather (HW ring limit is ~1.5k)
UPB = CAP // UNIT          # 5 gather units per bin
IDXU = UNIT // 16          # 64 idx columns per unit (16-partition wrap)
NUNITS = NB * UPB          # 20 gather units per core

_compiled = None


def _build(use_gather=True, use_cc=True):
    import concourse.bass as bass
    import concourse.mybir as mybir
    import concourse.tile as tile
    from concourse import bacc
    from concourse.bass import ts

    f32 = mybir.dt.float32
    nc = bacc.Bacc(None, target_bir_lowering=False, debug=False)

    x_in = nc.dram_tensor("x", [N, T], f32, kind="ExternalInput")
    w_in = nc.dram_tensor("wvec", [1, D], f32, kind="ExternalInput")
    iota_in = nc.dram_tensor("iota", [1, 128], f32, kind="ExternalInput")
    gidx_in = nc.dram_tensor("gidx", [128, NUNITS * IDXU], mybir.dt.int16,
                             kind="ExternalInput")
    dslot_in = nc.dram_tensor("dslot", [128, TOTCH], f32, kind="ExternalInput")
    dw_in = nc.dram_tensor("dw", [128, TOTCH], f32, kind="ExternalInput")
    sslot_in = nc.dram_tensor("sslot", [128, TOTCH], f32, kind="ExternalInput")
    sw_in = nc.dram_tensor("sw", [128, TOTCH], f32, kind="ExternalInput")
    out_dram = nc.dram_tensor("out", [BLK, T, D], f32, kind="ExternalOutput")

    y_dram = nc.dram_tensor("y_dram", [N, T], f32)
    ag_in = nc.dram_tensor("ag_in", [BLK, 1], f32)
    ag_out = nc.dram_tensor("ag_out", [N, 1], f32)

    with tile.TileContext(nc) as tc:
        with (
            tc.tile_pool(name="const", bufs=1) as constp,
            tc.tile_pool(name="edges", bufs=1) as edgep,
            tc.tile_pool(name="xy", bufs=1) as xyp,
            tc.tile_pool(name="mask", bufs=24) as maskp,
            tc.tile_pool(name="yg", bufs=4) as ygp,
            tc.tile_pool(name="dsm", bufs=1) as dsmp,
            tc.tile_pool(name="ep", bufs=3) as epp,
            tc.tile_pool(name="ep2", bufs=3) as ep2p,
            tc.tile_pool(name="psr", bufs=4, space="PSUM") as psr,
            tc.tile_pool(name="psm", bufs=4, space="PSUM") as psm,
        ):
            # constants
            iota_f = constp.tile([128, 128], f32)
            nc.sync.dma_start(out=iota_f[:], in_=iota_in[0:1, :].to_broadcast([128, 128]))
            wb = constp.tile([128, D], f32)
            nc.sync.dma_start(out=wb[:], in_=w_in[0:1, :].to_broadcast([128, D]))
            ones = constp.tile([128, 1], f32)
            nc.vector.memset(ones[:], 1.0)

            # edge metadata loads
            sslot_sb = edgep.tile([128, TOTCH], f32)
            nc.sync.dma_start(out=sslot_sb[:], in_=sslot_in[:, :])
            sw_sb = edgep.tile([128, TOTCH], f32)
            nc.sync.dma_start(out=sw_sb[:], in_=sw_in[:, :])
            dslot_sb = edgep.tile([128, TOTCH], f32)
            nc.sync.dma_start(out=dslot_sb[:], in_=dslot_in[:, :])
            dw_sb = edgep.tile([128, TOTCH], f32)
            nc.sync.dma_start(out=dw_sb[:], in_=dw_in[:, :])
            gidx_sb = edgep.tile([128, NUNITS * IDXU], mybir.dt.int16)
            nc.sync.dma_start(out=gidx_sb[:], in_=gidx_in[:, :])

            # full x: [128, 32*128], partition p holds rows {128k+p}
            x_sb = xyp.tile([128, N // 128, T], f32)
            nc.sync.dma_start(
                out=x_sb[:],
                in_=x_in[:, :].rearrange("(k p) t -> p k t", p=128),
            )

            # ---- rowsum of A over this core's 512 src rows ----
            rs_sb = dsmp.tile([128, NB], f32)
            for m in range(NB):
                ps = psr.tile([128, 1], f32, tag="rs")
                for ch in range(NCH):
                    col = m * NCH + ch
                    mask = maskp.tile([128, 128], f32, tag="mk")
                    nc.vector.tensor_scalar(
                        out=mask[:], in0=iota_f[:],
                        scalar1=sslot_sb[:, col:col + 1],
                        scalar2=sw_sb[:, col:col + 1],
                        op0=mybir.AluOpType.is_equal,
                        op1=mybir.AluOpType.mult,
                    )
                    nc.tensor.matmul(ps[:], lhsT=mask[:], rhs=ones[:],
                                     start=(ch == 0), stop=(ch == NCH - 1))
                nc.vector.tensor_copy(rs_sb[:, m:m + 1], ps[:])

            # d_local = 1/sqrt(rowsum)
            dl_s = dsmp.tile([128, NB], f32)
            nc.scalar.sqrt(dl_s[:], rs_sb[:])
            d_local = dsmp.tile([128, NB], f32)
            nc.vector.reciprocal(d_local[:], dl_s[:])

            # AllGather d: core c's block -> ag_out[512c : 512c+512]
            nc.sync.dma_start(
                out=ag_in[:, 0].rearrange("(m p) -> p m", p=128),
                in_=d_local[:],
            )
            if use_cc:
                nc.gpsimd.collective_compute(
                    "AllGather", mybir.AluOpType.bypass,
                    replica_groups=[list(range(NCORES))],
                    ins=[ag_in[:, :]], outs=[ag_out[:, :]],
                )
            else:
                nc.sync.dma_start(
                    out=ag_out[0:BLK, :],
                    in_=ag_in[:, :])
            d_sb = dsmp.tile([128, N // 128], f32)
            nc.sync.dma_start(
                out=d_sb[:],
                in_=ag_out[:, 0].rearrange("(k p) -> p k", p=128),
            )

            # y = d * x (row scaling), then to DRAM for the gathers
            y_sb = xyp.tile([128, N // 128, T], f32)
            for k in range(N // 128):
                nc.vector.tensor_scalar_mul(
                    y_sb[:, k, :], x_sb[:, k, :], d_sb[:, k:k + 1])
            nc.sync.dma_start(
                out=y_dram[:, :].rearrange("(k p) t -> p k t", p=128),
                in_=y_sb[:],
            )

            # per-bin gather of y rows + masked matmul + epilogue
            out_r = out_dram[:, :, :].rearrange("(m p) t d -> m p t d", p=128)
            for m in range(NB):
                pm = psm.tile([128, T], f32, tag="mm")
                for v in range(UPB):
                    u = m * UPB + v
                    yg = ygp.tile([128, UNIT // 128, T], f32, tag="yg")
                    if use_gather:
                        nc.gpsimd.dma_gather(
                            out_ap=yg[:],
                            in_ap=y_dram[:, :],
                            idxs_ap=gidx_sb[:, u * IDXU:(u + 1) * IDXU],
                            num_idxs=UNIT,
                            num_idxs_reg=UNIT,
                            elem_size=T,
                        )
                    else:
                        nc.vector.memset(yg[:], 0.0)
                    for i in range(UNIT // 128):
                        ch = v * (UNIT // 128) + i
                        col = m * NCH + ch
                        mask = maskp.tile([128, 128], f32, tag="mk")
                        nc.vector.tensor_scalar(
                            out=mask[:], in0=iota_f[:],
                            scalar1=dslot_sb[:, col:col + 1],
                            scalar2=dw_sb[:, col:col + 1],
                            op0=mybir.AluOpType.is_equal,
                            op1=mybir.AluOpType.mult,
                        )
                        nc.tensor.matmul(pm[:], lhsT=mask[:], rhs=yg[:, i, :],
                                         start=(ch == 0), stop=(ch == NCH - 1))
                # out[i, t, k] = tanh(Z[i, t] * w[k] * d_local[i])
                for tq in range(4):
                    prod = epp.tile([128, T // 4, D], f32, tag="ep")
                    nc.vector.tensor_tensor(
                        out=prod[:],
                        in0=pm[:, ts(tq, T // 4), None].to_broadcast(
                            [128, T // 4, D]),
                        in1=wb[:, None, :].to_broadcast([128, T // 4, D]),
                        op=mybir.AluOpType.mult,
                    )
                    tout = ep2p.tile([128, T // 4, D], f32, tag="ep2")
                    nc.scalar.activation(
                        tout[:], prod[:], mybir.ActivationFunctionType.Tanh,
                        scale=d_local[:, m:m + 1],
                    )
                    nc.sync.dma_start(
                        out=out_r[m, :, ts(tq, T // 4), :], in_=tout[:])
    nc.compile()
    return nc


def _get_compiled():
    global _compiled
    if _compiled is None:
        _compiled = _build()
    return _compiled


def _marshal(inputs, edge_index, edge_weight, weights):
    x = np.ascontiguousarray(np.asarray(inputs, dtype=np.float32))
    ei = np.asarray(edge_index)
    w = np.asarray(edge_weight, dtype=np.float32)
    wvec = np.ascontiguousarray(np.asarray(weights, dtype=np.float32))
    src = ei[0].astype(np.int64)
    dst = ei[1].astype(np.int64)

    # Dedup matching this backend's .at[src, dst].set(w) semantics: the
    # compiled scatter processes the edge list as 128 parallel chunks of
    # E/128, position-within-chunk ascending, so for duplicate (src, dst)
    # the edge with the largest (e mod E/128) wins (verified exhaustively
    # against the jax scatter for all 553 duplicate cells).
    E = len(w)
    mod = max(1, E // 128)
    e = np.arange(E)
    perm = np.lexsort((e, e % mod))
    key = (src * N + dst)[perm]
    _, first_rev = np.unique(key[::-1], return_index=True)
    keep = perm[E - 1 - first_rev]
    src, dst, w = src[keep], dst[keep], w[keep]

    # self-loops (adj + I) as ordinary edges; a real (i, i) edge coexists
    # with its self-loop and the matmul accumulation adds them: w + 1.
    loop = np.arange(N, dtype=np.int64)
    src = np.concatenate([src, loop])
    dst = np.concatenate([dst, loop])
    w = np.concatenate([w, np.ones(N, np.float32)])

    iota = np.arange(128, dtype=np.float32)[None, :]

    in_maps = []
    for c in range(NCORES):
        gidx = np.zeros((NB, CAP), np.int16)
        dslot = np.zeros((NB, CAP), np.float32)
        dwv = np.zeros((NB, CAP), np.float32)
        sslot = np.zeros((NB, CAP), np.float32)
        swv = np.zeros((NB, CAP), np.float32)
        for m in range(NB):
            lo = c * BLK + m * 128
            e = np.nonzero((dst >= lo) & (dst < lo + 128))[0]
            cnt = len(e)
            assert cnt <= CAP, f"dst bin overflow: {cnt} > {CAP}"
            gidx[m, :cnt] = src[e].astype(np.int16)
            dslot[m, :cnt] = (dst[e] - lo).astype(np.float32)
            dwv[m, :cnt] = w[e]
            es = np.nonzero((src >= lo) & (src < lo + 128))[0]
            cnt = len(es)
            assert cnt <= CAP, f"src bin overflow: {cnt} > {CAP}"
            sslot[m, :cnt] = (src[es] - lo).astype(np.float32)
            swv[m, :cnt] = w[es]

        # gather unit u = m*UPB + v covers entries [v*UNIT, (v+1)*UNIT) of
        # bin m; within a unit, entry e -> gidx_w[e % 16, u*IDXU + e // 16].
        # The 16-row block is replicated across all 8 GpSimd cores' groups.
        gidx_w = np.zeros((128, NUNITS * IDXU), np.int16)
        for m in range(NB):
            for v in range(UPB):
                u = m * UPB + v
                seg = gidx[m, v * UNIT:(v + 1) * UNIT]
                blk16 = seg.reshape(IDXU, 16).T
                gidx_w[:, u * IDXU:(u + 1) * IDXU] = np.tile(blk16, (8, 1))
        # entry e of bin m -> [e % 128, m*NCH + e // 128]
        def chunked(a):
            return np.ascontiguousarray(
                a.reshape(NB * NCH, 128).T.astype(np.float32))

        in_maps.append({
            "x": x,
            "wvec": wvec,
            "iota": iota,
            "gidx": np.ascontiguousarray(gidx_w),
            "dslot": chunked(dslot),
            "dw": chunked(dwv),
            "sslot": chunked(sslot),
            "sw": chunked(swv),
        })
    return in_maps


def kernel(inputs, edge_index, edge_weight, weights):
    from concourse.bass_utils import run_bass_kernel_spmd

    nc = _get_compiled()
    in_maps = _marshal(inputs, edge_index, edge_weight, weights)
    res = run_bass_kernel_spmd(nc, in_maps, core_ids=list(range(NCORES)))
    return np.concatenate([res.results[c]["out"] for c in range(NCORES)], axis=0)
